# revision 45
# baseline (speedup 1.0000x reference)
"""3-layer GAT on 8 trn2 NeuronCores.

Strategy (graph/data parallel per sharding hint):
  - Nodes are assigned to 8 cores x 49 blocks x 128 slots (degree-balanced
    LPT bin packing) -> permuted node order; "table row" = block*128 + slot.
  - Per layer: each core transforms its own node shard with
    rhs = [W | W@as | W@ad] (alpha terms folded into the matmul), writes a
    table shard [6272, F+2H(padded)], AllGather -> full table on every core.
  - Aggregation: per dst-block of 128 nodes, edges (dst-sorted) are packed
    into 128-edge tiles; a dma_gather fetches table rows for the tile's
    sources; a one-hot "scatter matrix" matmul accumulates the s_e-weighted
    [features | s] columns into PSUM (softmax denominator rides along as
    extra matmul columns).  (Softmax max-shift is skipped: logits are O(1)
    so exp is safe, and the result is mathematically identical.)
  - int16 gather indices: table split into lo rows [0,32768) and hi rows
    [17408,50176); per-block edges are balanced between the (overlapping)
    windows so each side fits 9 tiles of 128.
  - Engine split: gather + M one-hot build on GpSimd, MT build/z/tmp on
    DVE, exp/copies on Scalar, scatter matmuls on Tensor (bf16).
  - Layer 2 output is column-summed per core (masked for pad slots); the
    final mean + linear head run on host.
"""

import os
import numpy as np

# ---------------- problem constants (must match reference) ----------------
N = 50000
E = 800000
IN_C = 128
HID = 64
HEADS = 4
OUT_C = 64
F1 = HEADS * HID  # 256

# ---------------- sharding geometry ----------------
NCORES = 8
NB = 49           # dst blocks per core
BS = 128          # dst slots per block
NPC = NB * BS     # 6272 nodes per core
RTOT = NCORES * NPC  # 50176 table rows
TL = 9            # tiles per kind (lo/hi)
KE = TL * 128     # 1152 edge slots per (block, kind)
LO_LIM = 32768    # lo window rows [0, LO_LIM)
HI_BASE = 17408   # hi window rows [HI_BASE, HI_BASE+32768)
NKCOLS = KE // 16  # 72 idx columns per (block, kind)
# AllGather chunking: per-core block ranges; the small last chunk
# minimizes the exposed (un-overlapped) collective at layer boundaries.
AG_CHUNKS = [(0, 22), (22, 44), (44, 49)]
AG_SZ = [(e - s) * BS for s, e in AG_CHUNKS]          # rows/core per chunk
AG_PRE = [sum(AG_SZ[:i]) for i in range(len(AG_SZ))]  # rows/core before

USE_BF16 = os.environ.get("GAT_BF16", "1") == "1"

import ml_dtypes
TB_NP = ml_dtypes.bfloat16
EL01 = 384     # table elems/row layer0/1 (256 h + 4 as + 4 ad + pad)
EL2 = 128      # table elems/row layer2 (64 h + 1 as + 1 ad + pad)


# ---------------- host preprocessing ----------------

def preprocess(edge_index):
    """Node->(core,block,slot) assignment and per-core edge tile arrays.

    Returns dict with:
      row:   [N] table row of each node
      xperm: [RTOT] node id occupying each table row (-1 for pad slots)
      idx16: [NCORES,128,NB*2*NKCOLS] int16 wrapped gather indices
      dstc:  [NCORES,128,NB*2*TL] f32 dst_local per edge slot (col layout, -1 pad)
      dstr:  [NCORES,128,KE] f32 dst_local (row layout; partition=block*2+kind)
      maskc: [NCORES,128,NB] f32 1.0 for real-node slots
    """
    import heapq

    src = np.concatenate([np.asarray(edge_index[0]), np.arange(N, dtype=np.int64)])
    dst = np.concatenate([np.asarray(edge_index[1]), np.arange(N, dtype=np.int64)])
    deg = np.bincount(dst, minlength=N)

    nblocks = NCORES * NB
    order = np.argsort(-deg, kind="stable")
    heap = [(0, b) for b in range(nblocks)]
    heapq.heapify(heap)
    slots_used = np.zeros(nblocks, np.int64)
    node_block = np.empty(N, np.int64)
    node_slot = np.empty(N, np.int64)
    for n in order:
        popped = []
        while True:
            load, b = heapq.heappop(heap)
            if slots_used[b] < BS:
                break
            popped.append((load, b))
        node_block[n] = b
        node_slot[n] = slots_used[b]
        slots_used[b] += 1
        heapq.heappush(heap, (load + int(deg[n]), b))
        # blocks that were full stay out of the heap

    # table rows grouped by AllGather chunk: all cores' chunk-0 blocks
    # first, then all cores' chunk-1 blocks, etc.
    core_of = node_block // NB
    bl_of = node_block % NB
    row = np.zeros(N, np.int64)
    for ci, (s0, e0) in enumerate(AG_CHUNKS):
        m = (bl_of >= s0) & (bl_of < e0)
        row[m] = (NCORES * AG_PRE[ci] + core_of[m] * AG_SZ[ci]
                  + (bl_of[m] - s0) * BS + node_slot[m])

    xperm = np.full(RTOT, -1, np.int64)
    xperm[row] = np.arange(N)

    erow = row[src]          # gather row per edge
    eblk = node_block[dst]   # destination block per edge
    eslot = node_slot[dst]   # dst_local per edge

    idx16 = np.zeros((NCORES, 128, NB * 2 * NKCOLS), np.int16)
    kreal = np.zeros(NB * 2, np.int64)  # max real idx count per (block,kind)
    dstc = np.full((NCORES, 128, NB * 2 * TL), -1.0, np.float32)
    dstr = np.zeros((NCORES, 128, KE), np.float32)  # cast at build_core_inputs
    maskc = np.zeros((NCORES, 128, NB), np.float32)

    order_e = np.argsort(eblk, kind="stable")
    bounds = np.searchsorted(eblk[order_e], np.arange(nblocks + 1))

    for b in range(nblocks):
        c, bl = divmod(b, NB)
        es = order_e[bounds[b]:bounds[b + 1]]
        r_ = erow[es]
        dl = eslot[es]
        lo_f = r_ < HI_BASE
        hi_f = r_ >= LO_LIM
        flex = ~lo_f & ~hi_f
        n_lo = int(lo_f.sum())
        n_hi = int(hi_f.sum())
        n_fx = int(flex.sum())
        tot = n_lo + n_hi + n_fx
        assert tot <= 2 * KE, f"block {b} has {tot} edges > {2*KE}"
        # fill lo to capacity so hi usually needs one tile fewer
        add_lo = min(n_fx, KE - n_lo)
        if n_hi + (n_fx - add_lo) > KE:
            add_lo = n_fx - (KE - n_hi)
        assert 0 <= add_lo <= n_fx
        fx_idx = np.nonzero(flex)[0]
        sel_lo = np.zeros(len(es), bool)
        sel_lo[lo_f] = True
        sel_lo[fx_idx[:add_lo]] = True
        sel_hi = ~sel_lo
        assert sel_lo.sum() <= KE and sel_hi.sum() <= KE, (
            b, sel_lo.sum(), sel_hi.sum())

        for kind, sel, base in ((0, sel_lo, 0), (1, sel_hi, HI_BASE)):
            rr = r_[sel]
            dd = dl[sel]
            o = np.argsort(rr, kind="stable")  # DMA locality
            rr = rr[o]
            dd = dd[o]
            k = len(rr)
            assert k >= 1, (b, kind)
            # pad indices -1: dma_gather skips trailing negatives. The
            # first 8 (block,kind)s keep 0-pads so the gather ring
            # buffers never expose uninitialized SBUF to the exp().
            pad = 0 if bl * 2 + kind < 8 else -1
            rel = np.full(KE, pad, np.int64)
            rel[:k] = rr - base
            dloc = np.full(KE, -1.0, np.float32)
            dloc[:k] = dd.astype(np.float32)
            assert rel[:k].min() >= 0 and rel[:k].max() < 32768
            # wrapped idx: index i -> [i % 16, i // 16]
            w = rel.reshape(NKCOLS, 16).T.astype(np.int16)  # [16, NKCOLS]
            cbase = (bl * 2 + kind) * NKCOLS
            idx16[c, :, cbase:cbase + NKCOLS] = np.tile(w, (8, 1))
            # col layout: col bl*2*TL + kind*TL + t, partition p = edge t*128+p
            tcol = bl * 2 * TL + kind * TL
            dstc[c, :, tcol:tcol + TL] = dloc.reshape(TL, 128).T
            # row layout: partition bl*2+kind
            dstr[c, bl * 2 + kind, :] = dloc
            kreal[bl * 2 + kind] = max(kreal[bl * 2 + kind], k)

        # mask of real slots
        used = slots_used[b]
        maskc[c, :used, bl] = 1.0

    return dict(row=row, xperm=xperm, idx16=idx16, dstc=dstc, dstr=dstr,
                maskc=maskc, deg=deg, node_block=node_block,
                node_slot=node_slot, kreal=kreal)


def host_weights(inputs):
    """Extended weight matrices with folded attention vectors."""
    def ext(W, a_s, a_d, heads):
        # Was[k, h] = sum_c W[k, h*HID+c] * a_s[h, c]
        Wh = W.reshape(W.shape[0], heads, HID)
        Was = np.einsum("khc,hc->kh", Wh, a_s)
        Wad = np.einsum("khc,hc->kh", Wh, a_d)
        return np.concatenate([W, Was, Wad], axis=1).astype(np.float32)

    W0e = ext(np.asarray(inputs["W0"], np.float32),
              np.asarray(inputs["a0s"], np.float32),
              np.asarray(inputs["a0d"], np.float32), HEADS)      # [128, 264]
    W1e = ext(np.asarray(inputs["W1"], np.float32),
              np.asarray(inputs["a1s"], np.float32),
              np.asarray(inputs["a1d"], np.float32), HEADS)      # [256, 264]
    W2e = ext(np.asarray(inputs["W2"], np.float32),
              np.asarray(inputs["a2s"], np.float32),
              np.asarray(inputs["a2d"], np.float32), 1)          # [256, 66]
    return W0e, W1e, W2e


def build_core_inputs(inputs, pp):
    """Per-core in_maps for run_bass_kernel_spmd."""
    x = np.asarray(inputs["x"], np.float32)
    W0e, W1e, W2e = host_weights(inputs)
    b0 = np.asarray(inputs["b0"], np.float32)
    b1 = np.asarray(inputs["b1"], np.float32)
    b2 = np.asarray(inputs["b2"], np.float32)

    iota_row = np.tile(np.arange(128, dtype=np.float32), (128, 1))
    iota_col = np.arange(128, dtype=np.float32).reshape(128, 1)
    ones1 = np.ones((1, 128), TB_NP)
    ident = np.eye(128, dtype=np.float32)

    consts = dict(
        w0e=W0e.astype(TB_NP),                                  # [128, 264]
        w1e=W1e.reshape(2, 128, F1 + 2 * HEADS).astype(TB_NP),  # [2, 128, 264]
        w2e=W2e.reshape(2, 128, HID + 2).astype(TB_NP),         # [2, 128, 66]
        b0r=np.tile(b0, (128, 1)).astype(np.float32),
        b1r=np.tile(b1, (128, 1)).astype(np.float32),
        b2r=np.tile(b2, (128, 1)).astype(np.float32),
        iota_row=iota_row.astype(TB_NP), iota_col=iota_col,
        ones1=ones1, ident=ident.astype(TB_NP),
    )

    in_maps = []
    for c in range(NCORES):
        # xTb[b] = x[nodes of (c,b)].T : [128 feats, 128 slots]
        xtb = np.zeros((NB, IN_C, BS), np.float32)
        rows = np.concatenate([
            NCORES * AG_PRE[ci] + c * AG_SZ[ci] + np.arange(AG_SZ[ci])
            for ci in range(len(AG_CHUNKS))])
        nodes = pp["xperm"][rows].reshape(NB, BS)
        for b in range(NB):
            nb = nodes[b]
            valid = nb >= 0
            if valid.any():
                xtb[b][:, valid] = x[nb[valid]].T
        m = dict(
            xtb=xtb.astype(TB_NP),
            idx16=pp["idx16"][c],
            dstc=pp["dstc"][c].astype(TB_NP),
            dstr=pp["dstr"][c].astype(TB_NP),
            maskc=pp["maskc"][c],
            **consts,
        )
        in_maps.append(m)
    return in_maps


# ---------------- device kernel ----------------

_BUILT = None


def build_kernel(upto=99, pp=None):
    import concourse.bacc as bacc
    import concourse.bass as bass
    import concourse.mybir as mybir
    import concourse.tile as tile
    from concourse import library_config

    f32 = mybir.dt.float32
    tb_dt = mybir.dt.bfloat16
    i16 = mybir.dt.int16
    Alu = mybir.AluOpType
    Act = mybir.ActivationFunctionType

    nc = bacc.Bacc("TRN2", target_bir_lowering=False, debug=False,
                   num_devices=NCORES, num_swdge_queues=4)

    # ---- I/O ----
    xtb_d = nc.dram_tensor("xtb", [NB, IN_C, BS], tb_dt, kind="ExternalInput")
    idx16_d = nc.dram_tensor("idx16", [128, NB * 2 * NKCOLS], i16,
                             kind="ExternalInput")
    dstc_d = nc.dram_tensor("dstc", [128, NB * 2 * TL], tb_dt,
                            kind="ExternalInput")
    dstr_d = nc.dram_tensor("dstr", [128, KE], tb_dt, kind="ExternalInput")
    maskc_d = nc.dram_tensor("maskc", [128, NB], f32, kind="ExternalInput")
    w0e_d = nc.dram_tensor("w0e", [IN_C, F1 + 2 * HEADS], tb_dt,
                           kind="ExternalInput")
    w1e_d = nc.dram_tensor("w1e", [2, 128, F1 + 2 * HEADS], tb_dt,
                           kind="ExternalInput")
    w2e_d = nc.dram_tensor("w2e", [2, 128, HID + 2], tb_dt,
                           kind="ExternalInput")
    b0r_d = nc.dram_tensor("b0r", [128, F1], f32, kind="ExternalInput")
    b1r_d = nc.dram_tensor("b1r", [128, F1], f32, kind="ExternalInput")
    b2r_d = nc.dram_tensor("b2r", [128, HID], f32, kind="ExternalInput")
    iota_row_d = nc.dram_tensor("iota_row", [128, 128], tb_dt,
                                kind="ExternalInput")
    iota_col_d = nc.dram_tensor("iota_col", [128, 1], f32,
                                kind="ExternalInput")
    ones1_d = nc.dram_tensor("ones1", [1, 128], tb_dt, kind="ExternalInput")
    ident_d = nc.dram_tensor("ident", [128, 128], tb_dt, kind="ExternalInput")
    out_d = nc.dram_tensor("out_part", [1, OUT_C], f32, kind="ExternalOutput")

    # internal DRAM (chunk-split shards so a chunk's AllGather only
    # depends on / blocks the blocks it covers; per-layer tables so a
    # layer's AllGather can overlap the previous layer's aggregation)
    shards = [[nc.dram_tensor(f"shard{l}c{ci}",
                              [AG_SZ[ci], EL2 if l == 2 else EL01], tb_dt)
               for ci in range(len(AG_CHUNKS))] for l in range(3)]
    tables = [nc.dram_tensor(f"table{l}", [RTOT, EL2 if l == 2 else EL01],
                             tb_dt, addr_space="Shared") for l in range(3)]

    rg = [list(range(NCORES))]

    with tile.TileContext(nc) as tc:
        with (
            tc.tile_pool(name="const", bufs=1) as cpool,
            tc.tile_pool(name="big", bufs=1) as bigpool,
            tc.tile_pool(name="work", bufs=4) as wpool,
            tc.tile_pool(name="gather", bufs=6) as gpool,
            tc.tile_pool(name="small", bufs=4) as spool,
            tc.tile_pool(name="psum", bufs=2, space="PSUM") as ppool,
            tc.tile_pool(name="psum1", bufs=1, space="PSUM") as ppool1,
        ):
            # ---- load constants ----
            def load_const(tag, dram, shape, dtype=f32, view=None):
                t = cpool.tile(shape, dtype, tag=tag)
                nc.sync.dma_start(out=t[:], in_=view if view is not None
                                  else dram[:])
                return t

            w0e_s = load_const("w0e", w0e_d, [IN_C, F1 + 2 * HEADS], tb_dt)
            w1e_s = load_const("w1e", w1e_d, [128, 2, F1 + 2 * HEADS], tb_dt,
                               view=w1e_d[:].rearrange("c p j -> p c j"))
            w2e_s = load_const("w2e", w2e_d, [128, 2, HID + 2], tb_dt,
                               view=w2e_d[:].rearrange("c p j -> p c j"))
            b0r_s = load_const("b0r", b0r_d, [128, F1])
            b1r_s = load_const("b1r", b1r_d, [128, F1])
            b2r_s = load_const("b2r", b2r_d, [128, HID])
            iota_row_s = load_const("iota_row", iota_row_d, [128, 128], tb_dt)
            iota_col_s = load_const("iota_col", iota_col_d, [128, 1])
            ones1_s = load_const("ones1", ones1_d, [1, 128], tb_dt)
            ident_s = load_const("ident", ident_d, [128, 128], tb_dt)
            idx16_s = load_const("idx16", idx16_d,
                                 [128, NB * 2 * NKCOLS], i16)
            dstc_s = load_const("dstc", dstc_d, [128, NB * 2 * TL], tb_dt)
            dstr_s = load_const("dstr", dstr_d, [128, KE], tb_dt)
            maskc_s = load_const("maskc", maskc_d, [128, NB])

            nc.gpsimd.load_library(library_config.mlp)

            ad_alls = [cpool.tile([128, NB * h], tb_dt, tag=f"ad_all{i}",
                                  name=f"ad_all{i}")
                       for i, h in ((0, HEADS), (1, HEADS), (2, 1))]

            def transform_block(layer, b, hTb=None):
                """One block's transform -> shard DRAM + ad_all slice."""
                heads = 1 if layer == 2 else HEADS
                Fo = HID if layer == 2 else F1
                ncols = Fo + 2 * heads
                el = EL2 if layer == 2 else EL01
                ci = next(i for i, (s0, e0) in enumerate(AG_CHUNKS)
                          if s0 <= b < e0)
                shard = shards[layer][ci]
                srow = (b - AG_CHUNKS[ci][0]) * BS
                ad_all = ad_alls[layer]
                ps = ppool.tile([128, 512], f32, tag="tps", space="PSUM",
                                bufs=1)
                if layer == 0:
                    xb = wpool.tile([IN_C, BS], tb_dt, tag="xtb")
                    nc.sync.dma_start(out=xb[:], in_=xtb_d[b])
                    nc.tensor.matmul(out=ps[:, :ncols], lhsT=xb[:],
                                     rhs=w0e_s[:], start=True, stop=True)
                else:
                    we = w1e_s if layer == 1 else w2e_s
                    for k2 in range(2):
                        nc.tensor.matmul(
                            out=ps[:, :ncols],
                            lhsT=hTb[:, k2, :],
                            rhs=we[:, k2, :],
                            start=(k2 == 0), stop=(k2 == 1))
                tb = wpool.tile([128, el], tb_dt, tag="tbout")
                nc.scalar.activation(tb[:, :ncols], ps[:, :ncols],
                                     Act.Copy)
                nc.scalar.activation(
                    ad_all[:, b * heads:(b + 1) * heads],
                    ps[:, Fo + heads:Fo + 2 * heads], Act.Copy)
                nc.sync.dma_start(out=shard[srow:srow + BS, :], in_=tb[:])

            def transform(layer):
                for b in range(NB):
                    transform_block(layer, b)
                    for ci, (s0, e0) in enumerate(AG_CHUNKS[:-1]):
                        if b == e0 - 1:
                            allgather(layer, ci)
                return ad_alls[layer]

            def allgather(layer, ci):
                table = tables[layer]
                lo = NCORES * AG_PRE[ci]
                hi = lo + NCORES * AG_SZ[ci]
                nc.gpsimd.collective_compute(
                    "AllGather", mybir.AluOpType.bypass,
                    replica_groups=rg, ins=[shards[layer][ci][:].opt()],
                    outs=[table[lo:hi, :].opt()])

            def aggregate(layer, ad_all):
                heads = 1 if layer == 2 else HEADS
                Fo = HID if layer == 2 else F1
                el = EL2 if layer == 2 else EL01
                nct = Fo + heads
                table = tables[layer]
                brep = (b0r_s if layer == 0 else
                        (b1r_s if layer == 1 else b2r_s))
                views = [table[0:LO_LIM, :], table[HI_BASE:HI_BASE + 32768, :]]
                if layer == 2:
                    psum_sum = ppool1.tile([1, OUT_C], f32, tag="sum",
                                           space="PSUM")
                kreal = pp["kreal"]
                for b in range(NB):
                    pagg = ppool.tile([128, nct], f32, tag="agg",
                                      space="PSUM", bufs=3)
                    tls = [max(1, -(-int(kreal[b * 2 + kk]) // 128))
                           for kk in range(2)]
                    for kind in range(2):
                        bk = b * 2 + kind
                        tlk = tls[kind]
                        ke = tlk * 128
                        g = gpool.tile([128, TL, el], tb_dt, tag="g")
                        nc.gpsimd.dma_gather(
                            g[:, :tlk, :], views[kind],
                            idx16_s[:, bk * NKCOLS:bk * NKCOLS + tlk * 8],
                            ke, ke, el, single_packet=False,
                            queue_num=bk % 4)
                        # one-hot M [128e, tlk*128d]
                        M = wpool.tile([128, KE], tb_dt, tag="M", bufs=5)
                        tcol = b * 2 * TL + kind * TL
                        nc.vector.tensor_tensor(
                            out=M[:, :ke].rearrange("p (t d) -> p t d",
                                                    t=tlk),
                            in0=dstc_s[:, tcol:tcol + tlk].unsqueeze(-1)
                                .broadcast_to([128, tlk, 128]),
                            in1=iota_row_s[:].unsqueeze(1)
                                .broadcast_to([128, tlk, 128]),
                            op=Alu.is_equal)
                        # M_T [128d, tlk*128e] via replicated-row outer prod
                        MT = wpool.tile([128, KE], tb_dt, tag="MT", bufs=5)
                        dr = spool.tile([1, KE], tb_dt, tag="dr")
                        nc.sync.dma_start(out=dr[:, :ke],
                                          in_=dstr_d[bk:bk + 1, :ke])
                        for o in range(0, ke, 512):
                            wdt = min(512, ke - o)
                            pr = ppool1.tile([128, 512], f32, tag="rep",
                                             space="PSUM")
                            nc.tensor.matmul(out=pr[:, :wdt],
                                             lhsT=ones1_s[:],
                                             rhs=dr[:, o:o + wdt],
                                             start=True, stop=True)
                            nc.vector.tensor_tensor(
                                out=MT[:, o:o + wdt], in0=pr[:, :wdt],
                                in1=iota_col_s[:]
                                    .broadcast_to([128, wdt]),
                                op=Alu.is_equal)
                        # ad per edge via M_T @ ad_block
                        pad_ = ppool1.tile([128, TL * heads], f32, tag="adp",
                                           space="PSUM")
                        for t in range(tlk):
                            nc.tensor.matmul(
                                out=pad_[:, t * heads:(t + 1) * heads],
                                lhsT=MT[:, t * 128:(t + 1) * 128],
                                rhs=ad_all[:, b * heads:(b + 1) * heads],
                                start=True, stop=True)
                        # z = as + ad ; s = exp(max(z, 0.2 z)) -> tmp s-cols
                        z = spool.tile([128, TL * heads], f32, tag="z")
                        nc.vector.tensor_tensor(
                            out=z[:, :tlk * heads]
                                .rearrange("p (t h) -> p t h", t=tlk),
                            in0=g[:, :tlk, Fo:Fo + heads],
                            in1=pad_[:, :tlk * heads]
                                .rearrange("p (t h) -> p t h", t=tlk),
                            op=Alu.add)
                        zm = spool.tile([128, TL * heads], f32, tag="zm")
                        nc.scalar.activation(zm[:, :tlk * heads],
                                             z[:, :tlk * heads], Act.Prelu,
                                             alpha=0.2)
                        # tmp = [g_h * s | s] per tile: matmul rhs covers
                        # both the weighted features and the denominator.
                        tmp = wpool.tile([128, TL, nct], tb_dt, tag="tmp", bufs=5)
                        nc.scalar.activation(
                            tmp[:, :tlk, Fo:Fo + heads],
                            zm[:, :tlk * heads]
                                .rearrange("p (t h) -> p t h", t=tlk),
                            Act.Exp)
                        sv = tmp[:, :tlk, Fo:Fo + heads]
                        for hh in range(heads):
                            nc.vector.tensor_tensor(
                                out=tmp[:, :tlk, hh * HID:(hh + 1) * HID],
                                in0=g[:, :tlk, hh * HID:(hh + 1) * HID],
                                in1=sv[:, :, hh:hh + 1]
                                    .broadcast_to([128, tlk, HID]),
                                op=Alu.mult)
                        # accumulate [agg | den] in one matmul per tile
                        for t in range(tlk):
                            first = (kind == 0 and t == 0)
                            last = (kind == 1 and t == tlk - 1)
                            nc.tensor.matmul(
                                out=pagg[:],
                                lhsT=M[:, t * 128:(t + 1) * 128],
                                rhs=tmp[:, t, :],
                                start=first, stop=last)
                    # epilogue (+1e-16: pad slots have no edges -> den=0)
                    den = spool.tile([128, heads], f32, tag="den")
                    nc.vector.tensor_scalar(out=den[:],
                                            in0=pagg[:, Fo:Fo + heads],
                                            scalar1=1e-16, scalar2=None,
                                            op0=Alu.add)
                    rec = spool.tile([128, heads], f32, tag="rec")
                    nc.vector.reciprocal(out=rec[:], in_=den[:])
                    o1 = wpool.tile([128, Fo], f32, tag="o1")
                    for hh in range(heads):
                        nc.scalar.activation(
                            o1[:, hh * HID:(hh + 1) * HID],
                            pagg[:, hh * HID:(hh + 1) * HID],
                            Act.Copy, scale=rec[:, hh:hh + 1])
                    o2 = wpool.tile([128, Fo], f32, tag="o2")
                    nc.vector.tensor_tensor(out=o2[:], in0=o1[:],
                                            in1=brep[:, :Fo], op=Alu.add)
                    if layer == 2:
                        nc.tensor.matmul(out=psum_sum[:],
                                         lhsT=maskc_s[:, b:b + 1],
                                         rhs=o2[:], start=(b == 0),
                                         stop=(b == NB - 1))
                    else:
                        o3 = wpool.tile([128, Fo], tb_dt, tag="o3")
                        nc.scalar.activation(o3[:], o2[:], Act.Relu)
                        hTb = wpool.tile([128, 2, 128], tb_dt, tag="hTb")
                        for k2 in range(2):
                            pt = ppool1.tile([128, 128], tb_dt, tag="tp",
                                             space="PSUM")
                            nc.tensor.transpose(
                                pt[:], o3[:, k2 * 128:(k2 + 1) * 128],
                                ident_s[:])
                            nc.scalar.activation(hTb[:, k2, :], pt[:],
                                                 Act.Copy)
                        # next layer's transform for this block, inline:
                        # its tensor/scalar work overlaps the DVE/DMA-heavy
                        # aggregation of subsequent blocks, and each
                        # AllGather chunk fires as soon as its blocks are
                        # transformed (overlapping remaining aggregation).
                        transform_block(layer + 1, b, hTb)
                        for ci, (s0, e0) in enumerate(AG_CHUNKS[:-1]):
                            if b == e0 - 1:
                                allgather(layer + 1, ci)
                if layer < 2:
                    allgather(layer + 1, len(AG_CHUNKS) - 1)
                if layer == 2:
                    osb = spool.tile([1, OUT_C], f32, tag="osb")
                    nc.vector.tensor_copy(out=osb[:], in_=psum_sum[:])
                    nc.sync.dma_start(out=out_d[:], in_=osb[:])

            transform(0)
            allgather(0, len(AG_CHUNKS) - 1)
            for layer in range(3):
                # aggregate(l) also runs transform(l+1) + allgather(l+1)
                # chunks inline
                aggregate(layer, ad_alls[layer])

    nc.compile()
    return nc


def _get_built(pp=None):
    global _BUILT
    if _BUILT is None:
        _BUILT = build_kernel(upto=int(os.environ.get("GAT_UPTO", "99")),
                              pp=pp)
    return _BUILT


def kernel(**inputs) -> np.ndarray:
    from concourse.bass_utils import run_bass_kernel_spmd

    pp = preprocess(np.asarray(inputs["edge_index"]))
    in_maps = build_core_inputs(inputs, pp)
    nc = _get_built(pp)
    res = run_bass_kernel_spmd(nc, in_maps, core_ids=list(range(NCORES)))
    parts = np.stack([r["out_part"][0] for r in res.results])  # [8, 64]
    g = parts.sum(axis=0, keepdims=True) / N
    out = (g @ np.asarray(inputs["hw"], np.float32)
           + np.asarray(inputs["hb"], np.float32)).astype(np.float32)
    return out
